# revision 15
# baseline (speedup 1.0000x reference)
"""Trainium2 Bass kernel for BGraphConv (bidirectional GNN message passing).

Math (per layer, per direction):
    msg   = GELU(x[tgt] @ W1.T  +  (EA @ We.T + be) @ W2.T  +  (x @ Wn.T + bn)[srcd] @ W3.T + bm)
          = GELU(A[tgt] + B[srcd] + C_e)     after folding:
              A = x @ W1.T + b_msg           (node table, target side; all biases folded here)
              B = x @ (W3 @ Wn).T            (node table, source side)
              C = EA @ (W2 @ We).T           (edge matmul, on device)
    agg   = segment_mean(msg, tgt)
Then h = x + GELU(x@M1.T + fw@M2.T + bw@M3.T); x = LayerNorm(h)  (gamma=1, beta=0).

Distribution: edges partitioned by target-node range across 8 cores (fw: tgt=dst,
bw: tgt=src), sorted by target, padded per 128-target block to whole 128-edge
tiles.  Node tables are computed data-parallel over node shards and AllGathered
each layer.  Aggregation is fully local (no all-reduce): indicator matmuls
accumulate per-target-block sums in PSUM.  Pad edges point their B-gather at a
sentinel row (-100) so GELU drives their message to 0.
"""

import numpy as np

N = 32000
E = 256000
D = 128
NC = 8
NSH = N // NC            # nodes per shard
P = 128
NBLK = (NSH + P - 1) // P  # target blocks per shard
NLAYERS = 2
LN_EPS = 1e-5
SENT = -100.0


# ----------------------------------------------------------------------------
# Host-side preprocessing
# ----------------------------------------------------------------------------

def _fold_params(params):
    """Fold per-layer weights into device constants (numpy float32)."""
    out = []
    for lp in params:
        layer = {}
        tab_rhs = []
        for dr, key in enumerate(("fw", "bw")):
            p = lp[key]
            Wm = np.asarray(p["m"]["W"], np.float64)   # [D, 3D]
            bm = np.asarray(p["m"]["b"], np.float64)
            W1, W2, W3 = Wm[:, :D], Wm[:, D:2 * D], Wm[:, 2 * D:]
            Wn = np.asarray(p["n"]["W"], np.float64)
            bn = np.asarray(p["n"]["b"], np.float64)
            We = np.asarray(p["e"]["W"], np.float64)
            be = np.asarray(p["e"]["b"], np.float64)
            W_A = W1
            W_B = W3 @ Wn
            W_C = W2 @ We
            b_msg = bm + W2 @ be + W3 @ bn
            tab_rhs.append((W_A.T, W_B.T, b_msg))
            layer[f"wc_{dr}"] = np.ascontiguousarray(W_C.T, np.float32)
        # fused table rhs [128, 512]: A_fw | B_fw | A_bw | B_bw
        layer["tab_rhs"] = np.ascontiguousarray(
            np.concatenate([tab_rhs[0][0], tab_rhs[0][1],
                            tab_rhs[1][0], tab_rhs[1][1]], axis=1), np.float32)
        bias_blk = np.zeros((P, 4 * D), np.float64)
        bias_blk[:, 0:D] = tab_rhs[0][2][None, :]
        bias_blk[:, 2 * D:3 * D] = tab_rhs[1][2][None, :]
        layer["bias_blk"] = np.ascontiguousarray(bias_blk, np.float32)
        M = np.asarray(lp["merge_W"], np.float64)      # [D, 3D]
        for i in range(3):
            layer[f"m_{i}"] = np.ascontiguousarray(M[:, i * D:(i + 1) * D].T, np.float32)
        out.append(layer)
    return out


def _build_edge_structure(edge_index, edge_attr, n=N, nc_=NC):
    """Sort/shard/pad edges per direction. Returns per-core input arrays and the
    shared tile-count structure nt[dr][b]."""
    nsh = n // nc_
    nblk = (nsh + P - 1) // P
    src = np.asarray(edge_index[0], np.int64)
    dst = np.asarray(edge_index[1], np.int64)
    ea = np.asarray(edge_attr, np.float32)

    structure = []   # nt[dr][b]
    percore = [dict() for _ in range(nc_)]

    for dr in range(2):
        tgt = dst if dr == 0 else src
        oth = src if dr == 0 else dst
        # per (core, block) edge lists
        counts = np.zeros((nc_, nblk), np.int64)
        order = np.argsort(tgt, kind="stable")
        tgt_s, oth_s = tgt[order], oth[order]
        # block id for each sorted edge
        core_of = tgt_s // nsh
        blk_of = (tgt_s % nsh) // P
        for c in range(nc_):
            m = core_of == c
            bb = blk_of[m]
            counts[c] = np.bincount(bb, minlength=nblk)
        nt = np.maximum(1, (counts.max(axis=0) + P - 1) // P).astype(np.int64)
        structure.append(nt)
        ntot = int(nt.sum())          # tiles per core for this direction
        ep = ntot * P

        for c in range(nc_):
            m = core_of == c
            t_c, o_c = tgt_s[m], oth_s[m]
            e_c = order[m]            # original edge ids
            b_c = blk_of[m]
            # build padded arrays
            ea_t = np.zeros((P, ep), np.float32)
            aidx = np.zeros((P, ntot), np.int32)
            bidx = np.zeros((P, ntot), np.int32)      # pads point at row 0
            trel = np.zeros((P, ntot), np.float32)
            tpos = 0
            ptr = 0
            for b in range(nblk):
                k = int(counts[c, b])
                sl = slice(ptr, ptr + k)
                ptr += k
                nb = int(nt[b])
                # slot (tile j, partition p) for edge i in block: j = i // P, p = i % P
                jj = np.arange(k) // P
                pp = np.arange(k) % P
                cols = tpos + jj
                # feature-major EA: column = global edge slot s = (tpos+j)*P + p
                s = cols * P + pp
                ea_t[:, s] = ea[e_c[sl]].T
                aidx[pp, cols] = t_c[sl].astype(np.int32)
                bidx[pp, cols] = o_c[sl].astype(np.int32)
                trel[pp, cols] = (t_c[sl] - (c * nsh + b * P)).astype(np.float32)
                # pad slots: aidx points at block base (valid row), bidx sentinel
                base = c * nsh + b * P
                npad_slots = nb * P - k
                if npad_slots:
                    jj2 = (np.arange(k, nb * P)) // P
                    pp2 = (np.arange(k, nb * P)) % P
                    aidx[pp2, tpos + jj2] = base
                    trel[pp2, tpos + jj2] = -1.0
                tpos += nb
            # inverse counts per node  [128, nblk]
            node_cnt = np.bincount((t_c - c * nsh).astype(np.int64), minlength=nsh)
            inv = np.zeros(nblk * P, np.float32)
            inv[:nsh] = 1.0 / np.maximum(node_cnt, 1)
            invc = np.tile(inv[None, :], (P, 1))
            def wrap16(idx2d):
                # idx2d [P, ntot] int32, slot i = col*128 + p  ->  [128, ntot*8] int16
                flat = idx2d.T.reshape(-1)          # slot order
                ncols = flat.size // 16
                w = np.zeros((16, ncols), np.int16)
                i = np.arange(flat.size)
                w[i % 16, i // 16] = flat.astype(np.int16)
                return np.tile(w, (8, 1))
            d = percore[c]
            d[f"ea_{dr}"] = ea_t
            d[f"aidx_{dr}"] = wrap16(aidx)
            d[f"bidx_{dr}"] = wrap16(bidx)
            d[f"trel_{dr}"] = trel
            d[f"invc_{dr}"] = invc
    return percore, structure


def _prep_x(x, n=N, nc_=NC):
    nsh = n // nc_
    nblk = (nsh + P - 1) // P
    xs = np.asarray(x, np.float32)
    per = []
    for c in range(nc_):
        sh = xs[c * nsh:(c + 1) * nsh]                # [nsh, D]
        pad = np.zeros((nblk * P, D), np.float32)
        pad[:nsh] = sh
        x_nm = np.zeros((P, nblk * P), np.float32)    # [p, b*128+j] = x[b*128+p, j]
        for b in range(nblk):
            x_nm[:, b * P:(b + 1) * P] = pad[b * P:(b + 1) * P, :]
        x_t = np.ascontiguousarray(pad.T)             # [128 feat, nblk*128 nodes]
        per.append({"x_nm": x_nm, "x_t": x_t})
    return per


# ----------------------------------------------------------------------------
# Device program
# ----------------------------------------------------------------------------

def build_program(n, nc_, nt, consts, n_layers=NLAYERS, dbg=False):
    """Build the SPMD Bass program (same for all cores)."""
    from concourse import bacc, bass, mybir, tile

    nsh = n // nc_
    nblk = (nsh + P - 1) // P
    ntot = [int(x.sum()) for x in nt]
    f32 = mybir.dt.float32
    i32 = mybir.dt.int32

    nc = bacc.Bacc(None, target_bir_lowering=False, debug=False, num_devices=nc_,
                   dynamic_dma_scratch_size=32768)

    # I/O
    ins = {}
    for dr in range(2):
        ins[f"ea_{dr}"] = nc.dram_tensor(f"ea_{dr}", [P, ntot[dr] * P], f32, kind="ExternalInput")
        ins[f"aidx_{dr}"] = nc.dram_tensor(f"aidx_{dr}", [P, ntot[dr] * 8], mybir.dt.int16, kind="ExternalInput")
        ins[f"bidx_{dr}"] = nc.dram_tensor(f"bidx_{dr}", [P, ntot[dr] * 8], mybir.dt.int16, kind="ExternalInput")
        ins[f"trel_{dr}"] = nc.dram_tensor(f"trel_{dr}", [P, ntot[dr]], f32, kind="ExternalInput")
        ins[f"invc_{dr}"] = nc.dram_tensor(f"invc_{dr}", [P, nblk * P], f32, kind="ExternalInput")
    ins["x_t"] = nc.dram_tensor("x_t", [P, nblk * P], f32, kind="ExternalInput")
    ins["x_nm"] = nc.dram_tensor("x_nm", [P, nblk * P], f32, kind="ExternalInput")
    out_shard = nc.dram_tensor("out_shard", [nsh, D], f32, kind="ExternalOutput")
    if dbg:
        r0 = max(0, nsh - 100)
        dbg_tab = nc.dram_tensor("dbg_tab", [200, 4 * D], f32, kind="ExternalOutput")
        dbg_agg = [nc.dram_tensor(f"dbg_agg{i}", [P, nblk * P], f32, kind="ExternalOutput") for i in range(2)]
        dbg_x1 = nc.dram_tensor("dbg_x1", [P, nblk * P], f32, kind="ExternalOutput")
        dbg_gelu = nc.dram_tensor("dbg_gelu", [P, P], f32, kind="ExternalOutput")

    # internal DRAM
    table_shard = nc.dram_tensor("table_shard", [nsh, 4 * D], f32)
    table_full = nc.dram_tensor("table_full", [n, 4 * D], f32, addr_space="Shared")
    _tv = {}

    # inline consts
    ident_d = nc.inline_tensor(np.eye(P, dtype=np.float32), "ident")
    iota_d = nc.inline_tensor(np.tile(np.arange(P, dtype=np.float32), (P, 1)), "iota")
    eps_d = nc.inline_tensor(np.full((P, 1), LN_EPS, np.float32), "epsc")
    neg8_d = nc.inline_tensor(np.full((P, 1), -8.0, np.float32), "neg8c")
    cd = []
    for l in range(n_layers):
        lc = consts[l]
        cd.append({k: nc.inline_tensor(lc[k], f"c{l}_{k}")
                   for k in ("tab_rhs", "bias_blk", "wc_0", "wc_1", "m_0", "m_1", "m_2")})

    A_OFF = (0, 2 * D)
    B_OFF = (D, 3 * D)
    ins_tab_views = {}
    add_op = mybir.AluOpType.add
    iseq_op = mybir.AluOpType.is_equal
    AF = mybir.ActivationFunctionType

    for off in (0, D, 2 * D, 3 * D):
        ins_tab_views[off] = table_full[:, off:off + D]
    gsem_ctx = nc.semaphore("gsem")
    gsem = gsem_ctx.__enter__()
    gcnt = [0]
    with tile.TileContext(nc) as tc:
        import contextlib
        ctx = contextlib.ExitStack()
        with ctx:
            cpool = ctx.enter_context(tc.tile_pool(name="cpool", bufs=1))
            xpool = ctx.enter_context(tc.tile_pool(name="xpool", bufs=1))
            ldpool = ctx.enter_context(tc.tile_pool(name="ldpool", bufs=2))
            wpool = ctx.enter_context(tc.tile_pool(name="wpool", bufs=4))
            tabpool = ctx.enter_context(tc.tile_pool(name="tabpool", bufs=2))
            p_msg = ctx.enter_context(tc.tile_pool(name="p_msg", bufs=3, space="PSUM"))
            p_agg = ctx.enter_context(tc.tile_pool(name="p_agg", bufs=2, space="PSUM"))
            p_tab = ctx.enter_context(tc.tile_pool(name="p_tab", bufs=1, space="PSUM"))
            p_tr = ctx.enter_context(tc.tile_pool(name="p_tr", bufs=2, space="PSUM"))

            # persistent SBUF state
            def load_const(dram, shape):
                t = cpool.tile(shape, f32, tag=dram.name, name="c_" + dram.name)
                nc.sync.dma_start(out=t[:], in_=dram[:, :])
                return t

            ident = load_const(ident_d, [P, P])
            epsc = load_const(eps_d, [P, 1])
            neg8 = load_const(neg8_d, [P, 1])
            iota = load_const(iota_d, [P, P])
            csb = []
            for l in range(n_layers):
                csb.append({
                    "tab_rhs": load_const(cd[l]["tab_rhs"], [P, 4 * D]),
                    "bias_blk": load_const(cd[l]["bias_blk"], [P, 4 * D]),
                    "wc": [load_const(cd[l]["wc_0"], [P, P]),
                           load_const(cd[l]["wc_1"], [P, P])],
                    "m": [load_const(cd[l][f"m_{i}"], [P, P]) for i in range(3)],
                })

            x_t = xpool.tile([P, nblk * P], f32, tag="x_t")
            nc.sync.dma_start(out=x_t[:], in_=ins["x_t"][:, :])
            x_nm = xpool.tile([P, nblk * P], f32, tag="x_nm")
            nc.sync.dma_start(out=x_nm[:], in_=ins["x_nm"][:, :])
            aggT = [xpool.tile([P, nblk * P], f32, tag=f"aggT_{dr}", name=f"aggT_{dr}") for dr in range(2)]
            idx_sb = {}
            for dr in range(2):
                t = xpool.tile([P, ntot[dr]], f32, tag=f"trel_{dr}", name=f"trel_{dr}")
                nc.sync.dma_start(out=t[:], in_=ins[f"trel_{dr}"][:, :])
                idx_sb[f"trel_{dr}"] = t


            if dbg:
                gt = wpool.tile([P, P], f32, tag="mp", name="gt")
                nc.scalar.activation(gt[:], iota[:], AF.Gelu, scale=0.125, bias=neg8[:, 0:1])
                nc.sync.dma_start(out=dbg_gelu[:, :], in_=gt[:])

            for l in range(n_layers):
                lc = csb[l]
                # ---- table phase (own shard) ----
                for b in range(nblk):
                    pt = p_tab.tile([P, 4 * D], f32, tag="pt")
                    nc.tensor.matmul(pt[:], lhsT=x_t[:, b * P:(b + 1) * P],
                                     rhs=lc["tab_rhs"][:], start=True, stop=True)
                    ts = tabpool.tile([P, 4 * D], f32, tag="ts")
                    nc.vector.tensor_tensor(out=ts[:], in0=pt[:], in1=lc["bias_blk"][:], op=add_op)
                    rows = min(P, nsh - b * P)
                    nc.sync.dma_start(out=table_shard[b * P:b * P + rows, :], in_=ts[:rows, :])
                # ---- all-gather tables ----
                nc.gpsimd.collective_compute(
                    "AllGather", mybir.AluOpType.bypass,
                    replica_groups=[list(range(nc_))],
                    ins=[table_shard[:, :].opt()],
                    outs=[table_full[:, :].opt()],
                )
                # ---- edge phase ----
                for dr in range(2):
                    tbase = 0
                    GROUP = 2
                    for g0 in range(0, nblk, GROUP):
                        gbl = list(range(g0, min(g0 + GROUP, nblk)))
                        gnt = [int(nt[dr][b]) for b in gbl]
                        gtot = sum(gnt)
                        ea_blk = ldpool.tile([P, gtot * P], f32, tag="ea", name="ea_blk")
                        nc.sync.dma_start(
                            out=ea_blk[:],
                            in_=ins[f"ea_{dr}"][:, tbase * P:(tbase + gtot) * P])
                        ai = wpool.tile([P, gtot * 8], mybir.dt.int16, tag="ai", name="ai")
                        nc.sync.dma_start(out=ai[:], in_=ins[f"aidx_{dr}"][:, tbase * 8:(tbase + gtot) * 8])
                        bi2 = wpool.tile([P, gtot * 8], mybir.dt.int16, tag="bi2", name="bi2")
                        nc.sync.dma_start(out=bi2[:], in_=ins[f"bidx_{dr}"][:, tbase * 8:(tbase + gtot) * 8])
                        a_blk = ldpool.tile([P, gtot * P], f32, tag="agat", name="a_blk")
                        b_blk = ldpool.tile([P, gtot * P], f32, tag="bgat", name="b_blk")
                        nidx = gtot * P
                        with tc.tile_critical():
                            nc.gpsimd.dma_gather(
                                out_ap=a_blk[:].rearrange("p (g e) -> p g e", e=P),
                                in_ap=ins_tab_views[A_OFF[dr]],
                                idxs_ap=ai[:], num_idxs=nidx, num_idxs_reg=nidx,
                                elem_size=P, elem_step=4 * D,
                                single_packet=False).then_inc(gsem, 16)
                            nc.gpsimd.dma_gather(
                                out_ap=b_blk[:].rearrange("p (g e) -> p g e", e=P),
                                in_ap=ins_tab_views[B_OFF[dr]],
                                idxs_ap=bi2[:], num_idxs=nidx, num_idxs_reg=nidx,
                                elem_size=P, elem_step=4 * D,
                                single_packet=False).then_inc(gsem, 16)
                            gcnt[0] += 32
                            nc.gpsimd.wait_ge(gsem, gcnt[0])
                        icg = wpool.tile([P, len(gbl) * P], f32, tag="icg", name="icg")
                        nc.sync.dma_start(out=icg[:], in_=ins[f"invc_{dr}"][:, g0 * P:(g0 + len(gbl)) * P])
                        off = 0
                        for bi, b in enumerate(gbl):
                            nb = gnt[bi]
                            pa = p_agg.tile([P, P], f32, tag="pa", name="pa")
                            for j in range(nb):
                                t = tbase + off + j
                                jl = off + j
                                pm = p_msg.tile([P, P], f32, tag="pm", name="pm")
                                nc.tensor.matmul(pm[:], lhsT=ea_blk[:, jl * P:(jl + 1) * P],
                                                 rhs=lc["wc"][dr][:], start=True, stop=True)
                                mp0 = wpool.tile([P, P], f32, tag="mp0", name="mp0")
                                nc.vector.tensor_tensor(out=mp0[:], in0=a_blk[:, jl * P:(jl + 1) * P],
                                                        in1=b_blk[:, jl * P:(jl + 1) * P], op=add_op)
                                mp = wpool.tile([P, P], f32, tag="mp", name="mp")
                                nc.vector.tensor_tensor(out=mp[:], in0=mp0[:],
                                                        in1=pm[:], op=add_op)
                                ms = wpool.tile([P, P], f32, tag="ms", name="ms")
                                nc.scalar.activation(ms[:], mp[:], AF.Gelu)
                                ind = wpool.tile([P, P], f32, tag="ind", name="ind")
                                nc.vector.tensor_tensor(
                                    out=ind[:],
                                    in0=idx_sb[f"trel_{dr}"][:, t:t + 1].to_broadcast([P, P]),
                                    in1=iota[:], op=iseq_op)
                                nc.tensor.matmul(pa[:], lhsT=ms[:], rhs=ind[:],
                                                 start=(j == 0), stop=(j == nb - 1))
                            nc.vector.tensor_tensor(
                                out=aggT[dr][:, b * P:(b + 1) * P], in0=pa[:],
                                in1=icg[:, bi * P:(bi + 1) * P],
                                op=mybir.AluOpType.mult)
                            off += nb
                        tbase += gtot
                if dbg and l == 0:
                    dtb = ldpool.tile([P, 4 * D], f32, tag="ab", name="dtb")
                    nc.sync.dma_start(out=dtb[:], in_=table_full[r0:r0 + P, :])
                    nc.sync.dma_start(out=dbg_tab[0:P, :], in_=dtb[:])
                    dtb2 = ldpool.tile([72, 4 * D], f32, tag="ab2", name="dtb2")
                    nc.sync.dma_start(out=dtb2[:], in_=table_full[r0 + P:r0 + 200, :])
                    nc.sync.dma_start(out=dbg_tab[P:200, :], in_=dtb2[:])
                    for i in range(2):
                        nc.sync.dma_start(out=dbg_agg[i][:, :], in_=aggT[i][:])
                # ---- merge + LN phase ----
                for b in range(nblk):
                    cols = slice(b * P, (b + 1) * P)
                    pm = p_msg.tile([P, P], f32, tag="pm")
                    nc.tensor.matmul(pm[:], lhsT=x_t[:, cols], rhs=lc["m"][0][:],
                                     start=True, stop=False)
                    nc.tensor.matmul(pm[:], lhsT=aggT[0][:, cols], rhs=lc["m"][1][:],
                                     start=False, stop=False)
                    nc.tensor.matmul(pm[:], lhsT=aggT[1][:, cols], rhs=lc["m"][2][:],
                                     start=False, stop=True)
                    g = wpool.tile([P, P], f32, tag="g")
                    nc.scalar.activation(g[:], pm[:], AF.Gelu)
                    h = wpool.tile([P, P], f32, tag="h")
                    nc.vector.tensor_tensor(out=h[:], in0=g[:], in1=x_nm[:, cols], op=add_op)
                    s = wpool.tile([P, 1], f32, tag="s")
                    nc.vector.tensor_reduce(out=s[:], in_=h[:],
                                            axis=mybir.AxisListType.X, op=add_op)
                    nmu = wpool.tile([P, 1], f32, tag="nmu")
                    nc.scalar.activation(nmu[:], s[:], AF.Copy, scale=-1.0 / D)
                    ch = wpool.tile([P, P], f32, tag="ch")
                    nc.vector.tensor_scalar_add(out=ch[:], in0=h[:], scalar1=nmu[:, 0:1])
                    sq = wpool.tile([P, P], f32, tag="sq")
                    ssq = wpool.tile([P, 1], f32, tag="ssq")
                    nc.scalar.activation(sq[:], ch[:], AF.Square, accum_out=ssq[:])
                    stdv = wpool.tile([P, 1], f32, tag="stdv")
                    nc.scalar.activation(stdv[:], ssq[:], AF.Sqrt, scale=1.0 / D, bias=epsc[:, 0:1])
                    rst = wpool.tile([P, 1], f32, tag="rst")
                    nc.vector.reciprocal(out=rst[:], in_=stdv[:])
                    xn = wpool.tile([P, P], f32, tag="xn")
                    nc.vector.tensor_scalar_mul(out=xn[:], in0=ch[:], scalar1=rst[:, 0:1])
                    rows = min(P, nsh - b * P)
                    if l == n_layers - 1:
                        nc.sync.dma_start(out=out_shard[b * P:b * P + rows, :], in_=xn[:rows, :])
                    else:
                        nc.vector.tensor_copy(out=x_nm[:, cols], in_=xn[:])
                        if dbg:
                            nc.sync.dma_start(out=dbg_x1[:, cols], in_=xn[:])
                        pt2 = p_tr.tile([P, P], f32, tag="ptr")
                        nc.tensor.transpose(pt2[:], xn[:], ident[:])
                        nc.vector.tensor_copy(out=x_t[:, cols], in_=pt2[:])
    nc.finalize()
    return nc


# ----------------------------------------------------------------------------
# Entry point
# ----------------------------------------------------------------------------

def kernel(x, edge_index, edge_attr, params):
    from concourse.bass_utils import run_bass_kernel_spmd

    consts = _fold_params(params)
    percore, nt = _build_edge_structure(edge_index, edge_attr)
    perx = _prep_x(x)

    nc = build_program(N, NC, nt, consts)

    in_maps = []
    for c in range(NC):
        m = dict(percore[c])
        m.update(perx[c])
        in_maps.append(m)

    res = run_bass_kernel_spmd(nc, in_maps, core_ids=list(range(NC)))
    shards = [res.results[c]["out_shard"] for c in range(NC)]
    return np.concatenate(shards, axis=0).astype(np.float32)
